# revision 27
# baseline (speedup 1.0000x reference)
"""Trainium2 Bass kernel for spatial multi-head attention (gather-attention).

Computation (per agent b, H=8 heads, DK=32, K=32 neighbors, NB=16384):
    q = query @ Wq.T + bq ; k = query @ Wk.T (+bk) ; v = query @ Wv.T (+bv)
    s[b,h,k] = q[b,h,:] . k[nbr[b,k],h,:] / sqrt(DK)   (masked softmax over k)
    x[b,h,:] = sum_k p[b,h,k] v[nbr[b,k],h,:]
    out      = x @ Wo.T + bo

Key algebraic simplifications (exact):
  - bk drops out (softmax invariant to per-(b,h) constants: q.(k+bk) = q.k + const)
  - bv folds into the output bias: out = x0 @ Wo.T + (bo + Wo @ bv)

Device strategy (8 cores, data-parallel over agents, 2048 agents/core):
  - Each core redundantly projects the FULL fp16 K/V tables (PE matmuls) and
    writes them to its DRAM; gathers are served from DRAM at 512B/row.
  - Gather performance: dma_gather descgen runs on ONE Q7 core pair selected
    by queue_num (~8ns/idx). 4 SWDGE queues are enabled so the V gathers'
    descgen (queues 1-3) overlaps the K chain. K (transpose) gathers all stay
    on queue 0: their rx path goes through the shared stateful SBUF crossbar
    and two in-flight transpose gathers corrupt each other (verified on HW).
  - dynamic_dma_scratch_size is raised to 32KB: with the default 16KB ring
    carveout a single transpose gather (12288 descriptors) cannot fit two
    deep, and every K gather stalls the Pool engine head on inline ring
    drain (~1076us total vs ~755us with 32KB).
  - K gathered TRANSPOSED (d on partitions) -> score dot-products become
    PE partition-block reductions (lhsT = product slice, rhs = block-ones).
  - Scores land in [(b%4,k) partitions, (b//4, h) free] layout: softmax
    reduction over k is another PE block-reduce; normalization uses a PE
    4->128 partition broadcast.
  - V gathered NON-transposed with a host-side index permutation chosen so
    the normalized P tile aligns with V's layout with zero data movement,
    and the final output rows come out in natural agent order.
"""

import sys

if "/opt/trn_rl_repo" not in sys.path:
    sys.path.insert(0, "/opt/trn_rl_repo")

import numpy as np
from contextlib import ExitStack

H, DKD, DM = 8, 32, 256
KN = 32  # neighbors per agent
NB_FULL = 16384
NCORES = 8
SCALE = 1.0 / np.sqrt(DKD)
MASK_NEG = -1.0e9

_PROGRAM_CACHE = {}


def _build_program(NB, NBS, repeats=1, parts="all"):
    """Build the per-core Bass/Tile program. Identical across cores; all
    core-varying information comes in through input tensors."""
    import concourse.bacc as bacc
    import concourse.tile as tile
    import concourse.mybir as mybir
    from concourse.tile_rust import add_dep_helper
    from concourse.library_config import mlp as mlp_lib

    f16 = mybir.dt.float16
    f32 = mybir.dt.float32
    i16 = mybir.dt.int16
    Act = mybir.ActivationFunctionType

    CH = NBS // 128        # chunks of 128 agents
    NBT = NB // 128        # table row-tiles
    WG = min(8, NBT)       # table tiles batched per DRAM write
    IDXC = NBS * KN // 16  # idx tensor columns ([16, IDXC])

    nc = bacc.Bacc(
        "TRN2",
        target_bir_lowering=False,
        debug=False,
        num_swdge_queues=4,
        dynamic_dma_scratch_size=32768,
    )

    # ---- external inputs (host-prepped layouts) ----
    qT = nc.dram_tensor("qT", [DM, NB], f16, kind="ExternalInput").ap()
    qTs = nc.dram_tensor("qTs", [DM, NBS], f16, kind="ExternalInput").ap()
    WqT = nc.dram_tensor("WqT", [DM, DM], f16, kind="ExternalInput").ap()
    WkT = nc.dram_tensor("WkT", [DM, DM], f16, kind="ExternalInput").ap()
    WvT = nc.dram_tensor("WvT", [DM, DM], f16, kind="ExternalInput").ap()
    WoA = nc.dram_tensor("WoA", [DM, DM], f16, kind="ExternalInput").ap()
    bqv = nc.dram_tensor("bqv", [DM, 1], f32, kind="ExternalInput").ap()
    boeff = nc.dram_tensor("boeff", [1, DM], f16, kind="ExternalInput").ap()
    ones4 = nc.dram_tensor("ones4", [128, 4], f16, kind="ExternalInput").ap()
    onesT = nc.dram_tensor("onesT", [4, 128], f16, kind="ExternalInput").ap()
    ones1 = nc.dram_tensor("ones1", [1, 128], f16, kind="ExternalInput").ap()
    idxK = nc.dram_tensor("idxK", [128, IDXC], i16, kind="ExternalInput").ap()
    idxV = nc.dram_tensor("idxV", [128, IDXC], i16, kind="ExternalInput").ap()
    maskA = nc.dram_tensor("maskA", [128, CH * 32], f32, kind="ExternalInput").ap()
    outp = nc.dram_tensor("out", [NBS, DM], f32, kind="ExternalOutput").ap()

    # ---- internal DRAM K/V tables (fp16 rows, gather sources) ----
    ktab = nc.dram_tensor("ktab", [NB, DM], f16).ap()
    vtab = nc.dram_tensor("vtab", [NB, DM], f16).ap()

    with tile.TileContext(nc) as tc:
      for _rep in range(repeats):
        with ExitStack() as ctx:
            libload = nc.gpsimd.load_library(mlp_lib)
            consts = ctx.enter_context(tc.tile_pool(name="consts", bufs=1))

            def load_const(name, ap, shape, dtype, rearr=None):
                t = consts.tile(shape, dtype, tag=name)
                src = ap if rearr is None else ap.rearrange(rearr, p=128)
                nc.sync.dma_start(t[:], src)
                return t

            wq_sb = load_const("wq", WqT, [128, 2, DM], f16, "(c p) d -> p c d")
            wk_sb = load_const("wk", WkT, [128, 2, DM], f16, "(c p) d -> p c d")
            wv_sb = load_const("wv", WvT, [128, 2, DM], f16, "(c p) d -> p c d")
            wo_sb = load_const("wo", WoA, [128, 2, DM], f16, "(c p) d -> p c d")
            bq_sb = load_const("bq", bqv, [128, 2], f32, "(c p) o -> p (c o)")
            bo_sb = load_const("bo", boeff, [1, DM], f16)
            on4_sb = load_const("on4", ones4, [128, 4], f16)
            onT_sb = load_const("onT", onesT, [4, 128], f16)
            on1_sb = load_const("on1", ones1, [1, 128], f16)
            ixk_sb = load_const("ixk", idxK, [128, IDXC], i16)
            ixv_sb = load_const("ixv", idxV, [128, IDXC], i16)
            msk_sb = load_const("msk", maskA, [128, CH * 32], f32)

            qtn = ctx.enter_context(tc.tile_pool(name="qtn", bufs=1))
            qTn_sb = qtn.tile([128, 2, NBS], f16, tag="qTn")

            # ---------------- Phase A: projections ----------------
            with ExitStack() as actx:
                qpool = actx.enter_context(tc.tile_pool(name="qtp", bufs=1))
                qt_sb = qpool.tile([128, 2, NB], f16, tag="qt")
                nc.sync.dma_start(qt_sb[:], qT.rearrange("(c p) b -> p c b", p=128))
                qs_sb = qpool.tile([128, 2, NBS], f16, tag="qs")
                nc.sync.dma_start(qs_sb[:], qTs.rearrange("(c p) b -> p c b", p=128))

                aps = actx.enter_context(
                    tc.tile_pool(name="aps", bufs=2, space="PSUM")
                )
                astg = actx.enter_context(tc.tile_pool(name="astg", bufs=3))

                # q_T projection: qTn[d, b] = Wq @ qTs + bq  (fp16, d on parts)
                for dh in range(2):
                    for bc in range(NBS // 512):
                        qp = aps.tile([128, 512], f32, tag="qproj")
                        for ih in range(2):
                            nc.tensor.matmul(
                                qp[:],
                                lhsT=wq_sb[:, ih, dh * 128 : dh * 128 + 128],
                                rhs=qs_sb[:, ih, bc * 512 : bc * 512 + 512],
                                start=(ih == 0),
                                stop=(ih == 1),
                            )
                        nc.scalar.activation(
                            qTn_sb[:, dh, bc * 512 : bc * 512 + 512],
                            qp[:],
                            Act.Identity,
                            bias=bq_sb[:, dh : dh + 1],
                            scale=1.0,
                        )

                # K/V tables: tab[b, d] = qT[:, b].T @ W.T  (batched writes)
                # K pass first so K gathers (the serial transpose chain on
                # queue 0) can start as soon as possible; V tiles follow.
                kwr, vwr = [], []
                for g in range(NBT // WG):
                    kstg = astg.tile([128, WG, DM], f16, tag="kstg")
                    for j in range(WG):
                        bt = g * WG + j
                        kp = aps.tile([128, DM], f32, tag="kp")
                        for ih in range(2):
                            nc.tensor.matmul(
                                kp[:],
                                lhsT=qt_sb[:, ih, bt * 128 : bt * 128 + 128],
                                rhs=wk_sb[:, ih, :],
                                start=(ih == 0),
                                stop=(ih == 1),
                            )
                        nc.scalar.copy(kstg[:, j, :], kp[:])
                    rows = 128 * WG
                    kwr.append(
                        nc.sync.dma_start(
                            ktab[g * rows : (g + 1) * rows, :].rearrange(
                                "(j p) d -> p j d", p=128
                            ),
                            kstg[:],
                        )
                    )
                for g in range(NBT // WG):
                    vstg = astg.tile([128, WG, DM], f16, tag="vstg")
                    for j in range(WG):
                        bt = g * WG + j
                        vp = aps.tile([128, DM], f32, tag="vp")
                        for ih in range(2):
                            nc.tensor.matmul(
                                vp[:],
                                lhsT=qt_sb[:, ih, bt * 128 : bt * 128 + 128],
                                rhs=wv_sb[:, ih, :],
                                start=(ih == 0),
                                stop=(ih == 1),
                            )
                        nc.vector.tensor_copy(vstg[:, j, :], vp[:])
                    rows = 128 * WG
                    vwr.append(
                        nc.sync.dma_start(
                            vtab[g * rows : (g + 1) * rows, :].rearrange(
                                "(j p) d -> p j d", p=128
                            ),
                            vstg[:],
                        )
                    )

            # ---------------- Phase B: gather + attention chunks ----------------
            if parts == "phaseA":
                continue
            NIDXREG = 128 if parts == "gatherlite" else 4096
            kdma = [nc.alloc_semaphore("kdma0"), nc.alloc_semaphore("kdma1")]
            kgp = ctx.enter_context(tc.tile_pool(name="kgp", bufs=3))
            vgp = ctx.enter_context(tc.tile_pool(name="vgp", bufs=2))
            prp = ctx.enter_context(tc.tile_pool(name="prp", bufs=2))
            pvp = ctx.enter_context(tc.tile_pool(name="pvp", bufs=2))
            sfx = ctx.enter_context(tc.tile_pool(name="sfx", bufs=2))
            psc = ctx.enter_context(tc.tile_pool(name="psc", bufs=2, space="PSUM"))
            psz = ctx.enter_context(tc.tile_pool(name="psz", bufs=1, space="PSUM"))
            psr = ctx.enter_context(tc.tile_pool(name="psr", bufs=2, space="PSUM"))
            psx = ctx.enter_context(tc.tile_pool(name="psx", bufs=2, space="PSUM"))
            pso = ctx.enter_context(tc.tile_pool(name="pso", bufs=1, space="PSUM"))

            for ch in range(CH):
                icol = ch * 256  # 4096 idxs / 16 partitions

                # K gathers are TRANSPOSE gathers: their rx path goes through
                # the (stateful, shared) SBUF crossbar, so two K gathers must
                # never be in flight concurrently -> all on queue 0, which
                # serializes them. Non-transpose V gathers rotate over queues
                # 1-3 so their Q7 descriptor generation (on other core pairs)
                # overlaps the serial K chain.
                # K transpose-gathers: descgen runs on the Q7 pair of the
                # chosen queue and is the serial wall; alternate queues 0/1
                # so two descgens overlap. Their DMA transfers go through the
                # shared stateful SBUF crossbar and MUST stay exclusive:
                # prepare_only descgen, then a trigger gated (gpsimd wait_ge)
                # on the previous K transfer's DMA-completion sem. Consumers
                # additionally wait on the DMA sem (DVE wait_ge below) since
                # Tile ties a prep's output readiness to descgen completion.
                kq = ch % 2
                kg = kgp.tile([128, 2, 4096], f16, tag="kg")
                kgi = nc.gpsimd.dma_gather(
                    kg[:],
                    ktab,
                    ixk_sb[:, icol : icol + 256],
                    num_idxs=4096,
                    num_idxs_reg=NIDXREG,
                    elem_size=DM,
                    transpose=True,
                    single_packet=False,
                    prepare_only=True,
                    sem=kdma[kq],
                    queue_num=kq,
                )
                add_dep_helper(kgi.ins, libload.ins, sync=True)
                if ch > 0:
                    nc.gpsimd.wait_ge(kdma[(ch - 1) % 2], 16 * ((ch - 1) // 2 + 1))
                ktrig = nc.gpsimd.trigger_dma(count=None, queue_num=kq)
                for w in kwr:
                    add_dep_helper(ktrig.ins, w.ins, sync=True)

                vg = vgp.tile([128, KN, DM], f16, tag="vg")
                vgi = nc.gpsimd.dma_gather(
                    vg[:],
                    vtab,
                    ixv_sb[:, icol : icol + 256],
                    num_idxs=4096,
                    num_idxs_reg=NIDXREG,
                    elem_size=DM,
                    transpose=False,
                    single_packet=False,
                    queue_num=2 + ch % 2,
                )
                # Tile does not track the gathers' DRAM-source reads; order
                # them explicitly after the table writes (and the Q7 library
                # load that provides the gather handler).
                add_dep_helper(vgi.ins, libload.ins, sync=True)
                for w in vwr:
                    add_dep_helper(vgi.ins, w.ins, sync=True)

                # scores products: prod = Kg * q (q broadcast over k)
                # kg is filled by the prepared gather's DMA transfer; wait on
                # its completion sem explicitly (Tile only orders against the
                # prep's descgen tick).
                nc.vector.wait_ge(kdma[kq], 16 * (ch // 2 + 1))
                prod = prp.tile([128, 2, 4096], f16, tag="prod")
                qv = (
                    qTn_sb[:, :, ch * 128 : ch * 128 + 128]
                    .rearrange("p c (b u) -> p c b u", u=1)
                    .broadcast_to([128, 2, 128, KN])
                )
                nc.vector.tensor_mul(
                    prod[:].rearrange("p c (b k) -> p c b k", k=KN),
                    kg[:].rearrange("p c (b k) -> p c b k", k=KN),
                    qv,
                )

                # scores: PE block-reduce over d (32-partition blocks = heads)
                sc = psc.tile([128, KN, H], f32, tag="sc")
                for c in range(2):
                    for s in range(32):
                        nc.tensor.matmul(
                            sc[:, s, c * 4 : c * 4 + 4],
                            lhsT=prod[:, c, s * 128 : s * 128 + 128],
                            rhs=on4_sb[:, 0:4],
                            start=True,
                            stop=True,
                        )

                # masked softmax (un-normalized exp, then PE-normalize)
                sm = sfx.tile([128, KN, H], f32, tag="sm")
                mv = (
                    msk_sb[:, ch * 32 : ch * 32 + 32]
                    .rearrange("p (s u) -> p s u", u=1)
                    .broadcast_to([128, 32, H])
                )
                nc.vector.tensor_add(sm[:], sc[:], mv)
                ex = sfx.tile([128, KN, H], f16, tag="ex")
                nc.scalar.activation(ex[:], sm[:], Act.Exp, scale=float(SCALE))

                z = psz.tile([4, 256], f32, tag="z")
                nc.tensor.matmul(
                    z[:],
                    lhsT=on4_sb[:, 0:4],
                    rhs=ex[:].rearrange("p s h -> p (s h)"),
                    start=True,
                    stop=True,
                )
                rz = sfx.tile([4, 256], f32, tag="rz")
                nc.vector.reciprocal(rz[:], z[:])
                rz16 = sfx.tile([4, 256], f16, tag="rz16")
                nc.scalar.copy(rz16[:], rz[:])
                rb = psr.tile([128, 256], f32, tag="rb")
                nc.tensor.matmul(
                    rb[:], lhsT=onT_sb[:], rhs=rz16[:], start=True, stop=True
                )
                rb16 = sfx.tile([128, KN, H], f16, tag="rb16")
                nc.scalar.copy(rb16[:], rb[:].rearrange("p (s h) -> p s h", h=H))
                pn = sfx.tile([128, KN, H], f16, tag="pn")
                nc.vector.tensor_mul(pn[:], ex[:], rb16[:])

                # weighted values: prodv = Vg * P (P broadcast over dk)
                pv = pvp.tile([128, KN, DM], f16, tag="pv")
                nc.vector.tensor_mul(
                    pv[:].rearrange("p m (h d) -> p m h d", d=DKD),
                    vg[:].rearrange("p m (h d) -> p m h d", d=DKD),
                    pn[:]
                    .rearrange("p m (h u) -> p m h u", u=1)
                    .broadcast_to([128, KN, H, DKD]),
                )

                # x: PE block-reduce over k (32-partition blocks = agents)
                # layout [p, hh(d-half), m, a] so out-proj weights slice is
                # one contiguous free dim
                xp = psx.tile([128, 2, 32, 4], f32, tag="xp")
                pvf = pv[:].rearrange("p m d -> p (m d)")
                for s2 in range(64):
                    nc.tensor.matmul(
                        xp[:, s2 % 2, s2 // 2, :],
                        lhsT=pvf[:, s2 * 128 : s2 * 128 + 128],
                        rhs=on4_sb[:, 0:4],
                        start=True,
                        stop=True,
                    )
                x16 = sfx.tile([128, 2, 32, 4], f16, tag="x16")
                nc.scalar.copy(x16[:], xp[:])

                # output projection + bias
                op = pso.tile([128, DM], f32, tag="op")
                for c in range(2):
                    nc.tensor.matmul(
                        op[:],
                        lhsT=x16[:, c, :, :],
                        rhs=wo_sb[:, c, :],
                        start=(c == 0),
                        stop=False,
                        skip_group_check=True,
                    )
                nc.tensor.matmul(
                    op[:],
                    lhsT=on1_sb[:],
                    rhs=bo_sb[:],
                    start=False,
                    stop=True,
                    skip_group_check=True,
                )
                ou = sfx.tile([128, DM], f32, tag="ou")
                nc.vector.tensor_copy(ou[:], op[:])
                nc.sync.dma_start(outp[ch * 128 : ch * 128 + 128, :], ou[:])

    nc.compile()
    return nc


def _host_prep(query_, spatial_neighbors, mask, Wq, bq, Wk, bk, Wv, bv, Wo, bo,
               NB, NBS, ncores):
    """Pure-layout host prep: transposes, fp16 casts, index/mask relayout."""
    CH = NBS // 128
    f16 = np.float16

    q32 = np.asarray(query_, np.float32)
    qT16 = np.ascontiguousarray(q32.T).astype(f16)
    WqT16 = np.ascontiguousarray(np.asarray(Wq, np.float32).T).astype(f16)
    WkT16 = np.ascontiguousarray(np.asarray(Wk, np.float32).T).astype(f16)
    WvT16 = np.ascontiguousarray(np.asarray(Wv, np.float32).T).astype(f16)
    WoA16 = np.ascontiguousarray(np.asarray(Wo, np.float32).T).astype(f16)
    bq32 = np.asarray(bq, np.float32).reshape(DM, 1)
    boe = (np.asarray(bo, np.float64)
           + np.asarray(Wo, np.float64) @ np.asarray(bv, np.float64))
    boe16 = boe.astype(np.float32).astype(f16).reshape(1, DM)

    blk = (np.arange(128)[:, None] // 32 == np.arange(4)[None, :])
    ones4 = blk.astype(f16)
    onesT = np.ascontiguousarray(ones4.T)
    ones1 = np.ones((1, 128), f16)

    nbr = np.asarray(spatial_neighbors, np.int64)
    msk = np.asarray(mask, np.int32).reshape(NB, KN)

    def wrap16(flat):
        # flat index i at [i%16, i//16], replicated 8x for the 8 Q7 cores
        return np.tile(flat.reshape(-1, 16).T, (8, 1)).astype(np.int16)

    # V-gather permutation: i_local = m*128 + a*32 + k  ->  agent m*4+a, nbr k
    i_loc = np.arange(NBS * KN)
    chv = i_loc // 4096
    r = i_loc % 4096
    m_, a_, k_ = r // 128, (r % 128) // 32, r % 32
    bV = chv * 128 + m_ * 4 + a_

    # additive mask layout [ (a,k) partitions, (ch, s) ]: agent ch*128+s*4+a
    pa, pk = np.arange(128) // 32, np.arange(128) % 32
    chs = np.arange(CH * 32) // 32
    ss = np.arange(CH * 32) % 32

    per_core = []
    for c in range(ncores):
        base = c * NBS
        sl = slice(base, base + NBS)
        qTs16 = np.ascontiguousarray(q32[sl].T).astype(f16)

        nbr_c = nbr[sl]
        iK = wrap16(nbr_c.reshape(-1))  # order b*32+k
        iV = wrap16(nbr_c[bV, k_])      # permuted for V layout

        bM = chs[None, :] * 128 + ss[None, :] * 4 + pa[:, None]  # [128, CH*32]
        mA = np.where(msk[sl][bM, pk[:, None]] != 0, 0.0, MASK_NEG).astype(np.float32)

        per_core.append(
            dict(
                qT=qT16, qTs=qTs16, WqT=WqT16, WkT=WkT16, WvT=WvT16, WoA=WoA16,
                bqv=bq32, boeff=boe16, ones4=ones4, onesT=onesT, ones1=ones1,
                idxK=iK, idxV=iV, maskA=mA,
            )
        )
    return per_core


def kernel(**inputs):
    NB, NBS = NB_FULL, NB_FULL // NCORES
    key = (NB, NBS)
    if key not in _PROGRAM_CACHE:
        _PROGRAM_CACHE[key] = _build_program(NB, NBS)
    nc = _PROGRAM_CACHE[key]

    in_maps = _host_prep(NB=NB, NBS=NBS, ncores=NCORES, **inputs)

    from concourse.bass_utils import run_bass_kernel_spmd

    res = run_bass_kernel_spmd(nc, in_maps, list(range(NCORES)))
    out = np.concatenate([res.results[c]["out"] for c in range(NCORES)], axis=0)
    return out.reshape(NB, 1, DM).astype(np.float32)



# revision 29
# speedup vs baseline: 1.2887x; 1.2887x over previous
"""Trainium2 Bass kernel for spatial multi-head attention (gather-attention).

Computation (per agent b, H=8 heads, DK=32, K=32 neighbors, NB=16384):
    q = query @ Wq.T + bq ; k = query @ Wk.T (+bk) ; v = query @ Wv.T (+bv)
    s[b,h,k] = q[b,h,:] . k[nbr[b,k],h,:] / sqrt(DK)   (masked softmax over k)
    x[b,h,:] = sum_k p[b,h,k] v[nbr[b,k],h,:]
    out      = x @ Wo.T + bo

Key algebraic simplifications (exact):
  - bk drops out (softmax invariant to per-(b,h) constants: q.(k+bk) = q.k + const)
  - bv folds into the output bias: out = x0 @ Wo.T + (bo + Wo @ bv)

Device strategy (8 cores, data-parallel over agents, 2048 agents/core):
  - Each core redundantly projects the FULL fp16 K/V tables (PE matmuls) and
    writes them to its DRAM; gathers are served from DRAM at 512B/row.
  - Gather performance: dma_gather descgen runs on ONE Q7 core pair selected
    by queue_num (~8ns/idx). 4 SWDGE queues are enabled so the V gathers'
    descgen (queues 1-3) overlaps the K chain. K (transpose) gathers all stay
    on queue 0: their rx path goes through the shared stateful SBUF crossbar
    and two in-flight transpose gathers corrupt each other (verified on HW).
  - dynamic_dma_scratch_size is raised to 32KB: with the default 16KB ring
    carveout a single transpose gather (12288 descriptors) cannot fit two
    deep, and every K gather stalls the Pool engine head on inline ring
    drain (~1076us total vs ~755us with 32KB).
  - K gathered TRANSPOSED (d on partitions) -> score dot-products become
    PE partition-block reductions (lhsT = product slice, rhs = block-ones).
  - Scores land in [(b%4,k) partitions, (b//4, h) free] layout: softmax
    reduction over k is another PE block-reduce; normalization uses a PE
    4->128 partition broadcast.
  - V gathered NON-transposed with a host-side index permutation chosen so
    the normalized P tile aligns with V's layout with zero data movement,
    and the final output rows come out in natural agent order.
"""

import sys

if "/opt/trn_rl_repo" not in sys.path:
    sys.path.insert(0, "/opt/trn_rl_repo")

import numpy as np
from contextlib import ExitStack

H, DKD, DM = 8, 32, 256
KN = 32  # neighbors per agent
NB_FULL = 16384
NCORES = 8
SCALE = 1.0 / np.sqrt(DKD)
MASK_NEG = -1.0e9

_PROGRAM_CACHE = {}


def _build_program(NB, NBS, repeats=1, parts="all"):
    """Build the per-core Bass/Tile program. Identical across cores; all
    core-varying information comes in through input tensors."""
    import concourse.bacc as bacc
    import concourse.tile as tile
    import concourse.mybir as mybir
    from concourse.tile_rust import add_dep_helper
    from concourse.library_config import mlp as mlp_lib

    f16 = mybir.dt.float16
    f32 = mybir.dt.float32
    i16 = mybir.dt.int16
    Act = mybir.ActivationFunctionType

    CH = NBS // 128        # chunks of 128 agents
    NBT = NB // 128        # table row-tiles
    WG = min(8, NBT)       # table tiles batched per DRAM write
    IDXC = NBS * KN // 16  # idx tensor columns ([16, IDXC])

    nc = bacc.Bacc(
        "TRN2",
        target_bir_lowering=False,
        debug=False,
        num_swdge_queues=4,
        dynamic_dma_scratch_size=32768,
    )

    # ---- external inputs (host-prepped layouts) ----
    qT = nc.dram_tensor("qT", [DM, NB], f16, kind="ExternalInput").ap()
    qTs = nc.dram_tensor("qTs", [DM, NBS], f16, kind="ExternalInput").ap()
    WqT = nc.dram_tensor("WqT", [DM, DM], f16, kind="ExternalInput").ap()
    WkT = nc.dram_tensor("WkT", [DM, DM], f16, kind="ExternalInput").ap()
    WvT = nc.dram_tensor("WvT", [DM, DM], f16, kind="ExternalInput").ap()
    WoA = nc.dram_tensor("WoA", [DM, DM], f16, kind="ExternalInput").ap()
    bqv = nc.dram_tensor("bqv", [DM, 1], f32, kind="ExternalInput").ap()
    boeff = nc.dram_tensor("boeff", [1, DM], f16, kind="ExternalInput").ap()
    ones4 = nc.dram_tensor("ones4", [128, 4], f16, kind="ExternalInput").ap()
    onesT = nc.dram_tensor("onesT", [4, 128], f16, kind="ExternalInput").ap()
    ones1 = nc.dram_tensor("ones1", [1, 128], f16, kind="ExternalInput").ap()
    idxK = nc.dram_tensor("idxK", [128, IDXC], i16, kind="ExternalInput").ap()
    idxV = nc.dram_tensor("idxV", [128, IDXC], i16, kind="ExternalInput").ap()
    maskA = nc.dram_tensor("maskA", [128, CH * 32], f32, kind="ExternalInput").ap()
    outp = nc.dram_tensor("out", [NBS, DM], f32, kind="ExternalOutput").ap()

    # ---- internal DRAM K/V tables (fp16 rows, gather sources) ----
    ktab = nc.dram_tensor("ktab", [NB, DM], f16).ap()
    vtab = nc.dram_tensor("vtab", [NB, DM], f16).ap()

    with tile.TileContext(nc) as tc:
      for _rep in range(repeats):
        with ExitStack() as ctx:
            libload = nc.gpsimd.load_library(mlp_lib)
            consts = ctx.enter_context(tc.tile_pool(name="consts", bufs=1))

            def load_const(name, ap, shape, dtype, rearr=None):
                t = consts.tile(shape, dtype, tag=name)
                src = ap if rearr is None else ap.rearrange(rearr, p=128)
                nc.sync.dma_start(t[:], src)
                return t

            wq_sb = load_const("wq", WqT, [128, 2, DM], f16, "(c p) d -> p c d")
            wk_sb = load_const("wk", WkT, [128, 2, DM], f16, "(c p) d -> p c d")
            wv_sb = load_const("wv", WvT, [128, 2, DM], f16, "(c p) d -> p c d")
            wo_sb = load_const("wo", WoA, [128, 2, DM], f16, "(c p) d -> p c d")
            bq_sb = load_const("bq", bqv, [128, 2], f32, "(c p) o -> p (c o)")
            bo_sb = load_const("bo", boeff, [1, DM], f16)
            on4_sb = load_const("on4", ones4, [128, 4], f16)
            onT_sb = load_const("onT", onesT, [4, 128], f16)
            on1_sb = load_const("on1", ones1, [1, 128], f16)
            ixk_sb = load_const("ixk", idxK, [128, IDXC], i16)
            ixv_sb = load_const("ixv", idxV, [128, IDXC], i16)
            msk_sb = load_const("msk", maskA, [128, CH * 32], f32)

            qtn = ctx.enter_context(tc.tile_pool(name="qtn", bufs=1))
            qTn_sb = qtn.tile([128, 2, NBS], f16, tag="qTn")

            # ---------------- Phase A: projections ----------------
            with ExitStack() as actx:
                qpool = actx.enter_context(tc.tile_pool(name="qtp", bufs=1))
                qt_sb = qpool.tile([128, 2, NB], f16, tag="qt")
                nc.sync.dma_start(qt_sb[:], qT.rearrange("(c p) b -> p c b", p=128))
                qs_sb = qpool.tile([128, 2, NBS], f16, tag="qs")
                nc.sync.dma_start(qs_sb[:], qTs.rearrange("(c p) b -> p c b", p=128))

                aps = actx.enter_context(
                    tc.tile_pool(name="aps", bufs=2, space="PSUM")
                )
                astg = actx.enter_context(tc.tile_pool(name="astg", bufs=3))

                # q_T projection: qTn[d, b] = Wq @ qTs + bq  (fp16, d on parts)
                for dh in range(2):
                    for bc in range(NBS // 512):
                        qp = aps.tile([128, 512], f32, tag="qproj")
                        for ih in range(2):
                            nc.tensor.matmul(
                                qp[:],
                                lhsT=wq_sb[:, ih, dh * 128 : dh * 128 + 128],
                                rhs=qs_sb[:, ih, bc * 512 : bc * 512 + 512],
                                start=(ih == 0),
                                stop=(ih == 1),
                            )
                        nc.scalar.activation(
                            qTn_sb[:, dh, bc * 512 : bc * 512 + 512],
                            qp[:],
                            Act.Identity,
                            bias=bq_sb[:, dh : dh + 1],
                            scale=1.0,
                        )

                # K/V tables: tab[b, d] = qT[:, b].T @ W.T  (batched writes)
                # K pass first so K gathers (the serial transpose chain on
                # queue 0) can start as soon as possible; V tiles follow.
                kwr, vwr = [], []
                for g in range(NBT // WG):
                    kstg = astg.tile([128, WG, DM], f16, tag="kstg")
                    for j in range(0, WG, 2):
                        # two row-tiles per PSUM tile -> one double-width
                        # Act copy; halves the per-instruction overhead on
                        # the copy chain that gates the first K gather
                        kp = aps.tile([128, 2, DM], f32, tag="kp")
                        for u in range(2):
                            bt = g * WG + j + u
                            for ih in range(2):
                                nc.tensor.matmul(
                                    kp[:, u, :],
                                    lhsT=qt_sb[:, ih, bt * 128 : bt * 128 + 128],
                                    rhs=wk_sb[:, ih, :],
                                    start=(ih == 0),
                                    stop=(ih == 1),
                                )
                        nc.scalar.copy(kstg[:, j : j + 2, :], kp[:])
                    rows = 128 * WG
                    kwr.append(
                        nc.sync.dma_start(
                            ktab[g * rows : (g + 1) * rows, :].rearrange(
                                "(j p) d -> p j d", p=128
                            ),
                            kstg[:],
                        )
                    )
                for g in range(NBT // WG):
                    vstg = astg.tile([128, WG, DM], f16, tag="vstg")
                    for j in range(WG):
                        bt = g * WG + j
                        vp = aps.tile([128, DM], f32, tag="vp")
                        for ih in range(2):
                            nc.tensor.matmul(
                                vp[:],
                                lhsT=qt_sb[:, ih, bt * 128 : bt * 128 + 128],
                                rhs=wv_sb[:, ih, :],
                                start=(ih == 0),
                                stop=(ih == 1),
                            )
                        nc.vector.tensor_copy(vstg[:, j, :], vp[:])
                    rows = 128 * WG
                    vwr.append(
                        nc.sync.dma_start(
                            vtab[g * rows : (g + 1) * rows, :].rearrange(
                                "(j p) d -> p j d", p=128
                            ),
                            vstg[:],
                        )
                    )

            # ---------------- Phase B: gather + attention chunks ----------------
            if parts == "phaseA":
                continue
            NIDXREG = 128 if parts == "gatherlite" else 4096
            kgp = ctx.enter_context(tc.tile_pool(name="kgp", bufs=3))
            vgp = ctx.enter_context(tc.tile_pool(name="vgp", bufs=2))
            prp = ctx.enter_context(tc.tile_pool(name="prp", bufs=2))
            pvp = ctx.enter_context(tc.tile_pool(name="pvp", bufs=2))
            sfx = ctx.enter_context(tc.tile_pool(name="sfx", bufs=2))
            psc = ctx.enter_context(tc.tile_pool(name="psc", bufs=2, space="PSUM"))
            psz = ctx.enter_context(tc.tile_pool(name="psz", bufs=1, space="PSUM"))
            psr = ctx.enter_context(tc.tile_pool(name="psr", bufs=2, space="PSUM"))
            psx = ctx.enter_context(tc.tile_pool(name="psx", bufs=2, space="PSUM"))
            pso = ctx.enter_context(tc.tile_pool(name="pso", bufs=1, space="PSUM"))

            for ch in range(CH):
                icol = ch * 256  # 4096 idxs / 16 partitions

                # K gathers are TRANSPOSE gathers: their rx path goes through
                # the (stateful, shared) SBUF crossbar, so two K gathers must
                # never be in flight concurrently -> all on queue 0, which
                # serializes them. Non-transpose V gathers rotate over queues
                # 1-3 so their Q7 descriptor generation (on other core pairs)
                # overlaps the serial K chain.
                kg = kgp.tile([128, 2, 4096], f16, tag="kg")
                kgi = nc.gpsimd.dma_gather(
                    kg[:],
                    ktab,
                    ixk_sb[:, icol : icol + 256],
                    num_idxs=4096,
                    num_idxs_reg=NIDXREG,
                    elem_size=DM,
                    transpose=True,
                    single_packet=False,
                    queue_num=0,
                )
                vg = vgp.tile([128, KN, DM], f16, tag="vg")
                vgi = nc.gpsimd.dma_gather(
                    vg[:],
                    vtab,
                    ixv_sb[:, icol : icol + 256],
                    num_idxs=4096,
                    num_idxs_reg=NIDXREG,
                    elem_size=DM,
                    transpose=False,
                    single_packet=False,
                    queue_num=1 + ch % 3,
                )
                # Tile does not track the gathers' DRAM-source reads; order
                # them explicitly after the table writes (and the Q7 library
                # load that provides the gather handler).
                add_dep_helper(kgi.ins, libload.ins, sync=True)
                add_dep_helper(vgi.ins, libload.ins, sync=True)
                for w in kwr:
                    add_dep_helper(kgi.ins, w.ins, sync=True)
                for w in vwr:
                    add_dep_helper(vgi.ins, w.ins, sync=True)

                # scores products: prod = Kg * q (q broadcast over k)
                prod = prp.tile([128, 2, 4096], f16, tag="prod")
                qv = (
                    qTn_sb[:, :, ch * 128 : ch * 128 + 128]
                    .rearrange("p c (b u) -> p c b u", u=1)
                    .broadcast_to([128, 2, 128, KN])
                )
                nc.vector.tensor_mul(
                    prod[:].rearrange("p c (b k) -> p c b k", k=KN),
                    kg[:].rearrange("p c (b k) -> p c b k", k=KN),
                    qv,
                )

                # scores: PE block-reduce over d (32-partition blocks = heads)
                sc = psc.tile([128, KN, H], f32, tag="sc")
                for c in range(2):
                    for s in range(32):
                        nc.tensor.matmul(
                            sc[:, s, c * 4 : c * 4 + 4],
                            lhsT=prod[:, c, s * 128 : s * 128 + 128],
                            rhs=on4_sb[:, 0:4],
                            start=True,
                            stop=True,
                        )

                # masked softmax (un-normalized exp, then PE-normalize)
                sm = sfx.tile([128, KN, H], f32, tag="sm")
                mv = (
                    msk_sb[:, ch * 32 : ch * 32 + 32]
                    .rearrange("p (s u) -> p s u", u=1)
                    .broadcast_to([128, 32, H])
                )
                nc.vector.tensor_add(sm[:], sc[:], mv)
                ex = sfx.tile([128, KN, H], f16, tag="ex")
                nc.scalar.activation(ex[:], sm[:], Act.Exp, scale=float(SCALE))

                z = psz.tile([4, 256], f32, tag="z")
                nc.tensor.matmul(
                    z[:],
                    lhsT=on4_sb[:, 0:4],
                    rhs=ex[:].rearrange("p s h -> p (s h)"),
                    start=True,
                    stop=True,
                )
                rz = sfx.tile([4, 256], f32, tag="rz")
                nc.vector.reciprocal(rz[:], z[:])
                rz16 = sfx.tile([4, 256], f16, tag="rz16")
                nc.scalar.copy(rz16[:], rz[:])
                rb = psr.tile([128, 256], f32, tag="rb")
                nc.tensor.matmul(
                    rb[:], lhsT=onT_sb[:], rhs=rz16[:], start=True, stop=True
                )
                rb16 = sfx.tile([128, KN, H], f16, tag="rb16")
                nc.scalar.copy(rb16[:], rb[:].rearrange("p (s h) -> p s h", h=H))
                pn = sfx.tile([128, KN, H], f16, tag="pn")
                nc.vector.tensor_mul(pn[:], ex[:], rb16[:])

                # weighted values: prodv = Vg * P (P broadcast over dk)
                pv = pvp.tile([128, KN, DM], f16, tag="pv")
                nc.vector.tensor_mul(
                    pv[:].rearrange("p m (h d) -> p m h d", d=DKD),
                    vg[:].rearrange("p m (h d) -> p m h d", d=DKD),
                    pn[:]
                    .rearrange("p m (h u) -> p m h u", u=1)
                    .broadcast_to([128, KN, H, DKD]),
                )

                # x: PE block-reduce over k (32-partition blocks = agents)
                # layout [p, hh(d-half), m, a] so out-proj weights slice is
                # one contiguous free dim
                xp = psx.tile([128, 2, 32, 4], f32, tag="xp")
                pvf = pv[:].rearrange("p m d -> p (m d)")
                for s2 in range(64):
                    nc.tensor.matmul(
                        xp[:, s2 % 2, s2 // 2, :],
                        lhsT=pvf[:, s2 * 128 : s2 * 128 + 128],
                        rhs=on4_sb[:, 0:4],
                        start=True,
                        stop=True,
                    )
                x16 = sfx.tile([128, 2, 32, 4], f16, tag="x16")
                nc.scalar.copy(x16[:], xp[:])

                # output projection + bias
                op = pso.tile([128, DM], f32, tag="op")
                for c in range(2):
                    nc.tensor.matmul(
                        op[:],
                        lhsT=x16[:, c, :, :],
                        rhs=wo_sb[:, c, :],
                        start=(c == 0),
                        stop=False,
                        skip_group_check=True,
                    )
                nc.tensor.matmul(
                    op[:],
                    lhsT=on1_sb[:],
                    rhs=bo_sb[:],
                    start=False,
                    stop=True,
                    skip_group_check=True,
                )
                ou = sfx.tile([128, DM], f32, tag="ou")
                nc.vector.tensor_copy(ou[:], op[:])
                nc.sync.dma_start(outp[ch * 128 : ch * 128 + 128, :], ou[:])

    nc.compile()
    return nc


def _host_prep(query_, spatial_neighbors, mask, Wq, bq, Wk, bk, Wv, bv, Wo, bo,
               NB, NBS, ncores):
    """Pure-layout host prep: transposes, fp16 casts, index/mask relayout."""
    CH = NBS // 128
    f16 = np.float16

    q32 = np.asarray(query_, np.float32)
    qT16 = np.ascontiguousarray(q32.T).astype(f16)
    WqT16 = np.ascontiguousarray(np.asarray(Wq, np.float32).T).astype(f16)
    WkT16 = np.ascontiguousarray(np.asarray(Wk, np.float32).T).astype(f16)
    WvT16 = np.ascontiguousarray(np.asarray(Wv, np.float32).T).astype(f16)
    WoA16 = np.ascontiguousarray(np.asarray(Wo, np.float32).T).astype(f16)
    bq32 = np.asarray(bq, np.float32).reshape(DM, 1)
    boe = (np.asarray(bo, np.float64)
           + np.asarray(Wo, np.float64) @ np.asarray(bv, np.float64))
    boe16 = boe.astype(np.float32).astype(f16).reshape(1, DM)

    blk = (np.arange(128)[:, None] // 32 == np.arange(4)[None, :])
    ones4 = blk.astype(f16)
    onesT = np.ascontiguousarray(ones4.T)
    ones1 = np.ones((1, 128), f16)

    nbr = np.asarray(spatial_neighbors, np.int64)
    msk = np.asarray(mask, np.int32).reshape(NB, KN)

    def wrap16(flat):
        # flat index i at [i%16, i//16], replicated 8x for the 8 Q7 cores
        return np.tile(flat.reshape(-1, 16).T, (8, 1)).astype(np.int16)

    # V-gather permutation: i_local = m*128 + a*32 + k  ->  agent m*4+a, nbr k
    i_loc = np.arange(NBS * KN)
    chv = i_loc // 4096
    r = i_loc % 4096
    m_, a_, k_ = r // 128, (r % 128) // 32, r % 32
    bV = chv * 128 + m_ * 4 + a_

    # additive mask layout [ (a,k) partitions, (ch, s) ]: agent ch*128+s*4+a
    pa, pk = np.arange(128) // 32, np.arange(128) % 32
    chs = np.arange(CH * 32) // 32
    ss = np.arange(CH * 32) % 32

    per_core = []
    for c in range(ncores):
        base = c * NBS
        sl = slice(base, base + NBS)
        qTs16 = np.ascontiguousarray(q32[sl].T).astype(f16)

        nbr_c = nbr[sl]
        iK = wrap16(nbr_c.reshape(-1))  # order b*32+k
        iV = wrap16(nbr_c[bV, k_])      # permuted for V layout

        bM = chs[None, :] * 128 + ss[None, :] * 4 + pa[:, None]  # [128, CH*32]
        mA = np.where(msk[sl][bM, pk[:, None]] != 0, 0.0, MASK_NEG).astype(np.float32)

        per_core.append(
            dict(
                qT=qT16, qTs=qTs16, WqT=WqT16, WkT=WkT16, WvT=WvT16, WoA=WoA16,
                bqv=bq32, boeff=boe16, ones4=ones4, onesT=onesT, ones1=ones1,
                idxK=iK, idxV=iV, maskA=mA,
            )
        )
    return per_core


def kernel(**inputs):
    NB, NBS = NB_FULL, NB_FULL // NCORES
    key = (NB, NBS)
    if key not in _PROGRAM_CACHE:
        _PROGRAM_CACHE[key] = _build_program(NB, NBS)
    nc = _PROGRAM_CACHE[key]

    in_maps = _host_prep(NB=NB, NBS=NBS, ncores=NCORES, **inputs)

    from concourse.bass_utils import run_bass_kernel_spmd

    res = run_bass_kernel_spmd(nc, in_maps, list(range(NCORES)))
    out = np.concatenate([res.results[c]["out"] for c in range(NCORES)], axis=0)
    return out.reshape(NB, 1, DM).astype(np.float32)

